# revision 8
# baseline (speedup 1.0000x reference)
"""Trainium2 Bass kernel for BART custom-mask attention.

Problem: B=4, T=S=1024, E=1024, H=16 heads, D=64.
  q = (hs @ q_w.T + q_b) * D**-0.5 ; k/v analogous
  scores = q k^T + attention_mask ; attn = softmax(scores)
  attn(head h) *= (1-hm[h]) + hm[h]*(relation_inputs>0)   (no renorm)
  out = (attn @ v) @ o_w.T + o_b

Sharding: 8 cores = batch (4) x head-group (2, 8 heads each). Each core
computes a 512-feature slice of the attention output and projects it
through the matching o_w columns; the host sums the two half-partials
per batch.

On-core layout (bf16 compute, fp32 accumulation):
  - projections consume host-pre-transposed xT/wT so the contraction dim
    (e) is always on partitions; q/k are produced transposed [f, t],
    v natural [s, f].
  - scoresT [s, t] per head via 64x128 row-tiled PE (two heads of a pair
    run on independent half-arrays T0/T8).
  - exp on ScalarE straight out of PSUM -> bf16 E tiles.
  - denominators = ones[128,64]^T @ E via 128x64 column-tiled PE: both
    heads of a pair land in one [128,1024] PSUM tile with the per-head
    row sums replicated across 64 partitions -- exactly the broadcast the
    normalization multiply needs.
  - attn@v column-tiled the same way -> outT [f(128), t] PSUM per pair;
    one reciprocal_approx_fast + one tensor_mul does normalize+copyback.
  - output projection contracts f (on partitions) -> y [t, e] natural.
"""

import os
import sys

import numpy as np

for _p in ("/opt/trn_rl_repo", "/root/.axon_site/_ro/trn_rl_repo"):
    if os.path.isdir(_p) and _p not in sys.path:
        sys.path.insert(0, _p)
        break

import ml_dtypes

B, T, E, H = 4, 1024, 1024, 16
D = E // H
SCALING = D ** -0.5
N_CORES = 8
FH = 512          # features per core (8 heads x 64)
P = 128
BF16 = ml_dtypes.bfloat16

_PROGS = {}


def _build_program(mask_on, slot_flags):
    import concourse.tile as tile
    from concourse import bacc, mybir
    from contextlib import ExitStack

    bf = mybir.dt.bfloat16
    f32 = mybir.dt.float32
    Exp = mybir.ActivationFunctionType.Exp

    nc = bacc.Bacc("TRN2", target_bir_lowering=False, debug=False,
                   num_devices=N_CORES)

    xT_d = nc.declare_dram_parameter("xT", [E, T], bf, isOutput=False)
    wqT_d = nc.declare_dram_parameter("wqT", [E, FH], bf, isOutput=False)
    wkT_d = nc.declare_dram_parameter("wkT", [E, FH], bf, isOutput=False)
    wvT_d = nc.declare_dram_parameter("wvT", [E, FH], bf, isOutput=False)
    owT_d = nc.declare_dram_parameter("owT", [FH, E], bf, isOutput=False)
    qb_d = nc.declare_dram_parameter("qb", [P, 4], f32, isOutput=False)
    kb_d = nc.declare_dram_parameter("kb", [P, 4], f32, isOutput=False)
    vbb_d = nc.declare_dram_parameter("vbb", [P, FH], f32, isOutput=False)
    obb_d = nc.declare_dram_parameter("obb", [P, E], f32, isOutput=False)
    relM_d = {}
    for k in range(8):
        if slot_flags[k]:
            relM_d[k] = nc.declare_dram_parameter(f"relM{k}", [T, T], bf,
                                                  isOutput=False)
    if mask_on:
        expm_d = nc.declare_dram_parameter("expmaskT", [T, T], bf,
                                           isOutput=False)
    y_d = nc.declare_dram_parameter("y", [T, E], f32, isOutput=True)

    with tile.TileContext(nc) as tc, ExitStack() as ctx:
        persist = ctx.enter_context(tc.tile_pool(name="persist", bufs=1))

        # Spread input loads across the three DMA-capable engines (sync/
        # scalar HWDGE + gpsimd SWDGE) so they don't serialize on one
        # queue, and order them so the QKV-critical chunks land first.
        dma_engines = [nc.sync, nc.scalar, nc.gpsimd]
        dma_rr = [0]

        def dma_in(out_ap, in_ap):
            eng = dma_engines[dma_rr[0] % len(dma_engines)]
            dma_rr[0] += 1
            eng.dma_start(out_ap, in_ap)

        def alloc_tiles(n, rows, cols, nm, dt=bf):
            return [persist.tile([rows, cols], dt, name=f"{nm}{i}",
                                 tag=f"{nm}{i}") for i in range(n)]

        def load_chunk(ts, dram, rows, i):
            dma_in(ts[i][:], dram[rows * i:rows * (i + 1), :])

        xT_t = alloc_tiles(8, P, T, "xTt")
        wqT_t = alloc_tiles(8, P, FH, "wqTt")
        wkT_t = alloc_tiles(8, P, FH, "wkTt")
        wvT_t = alloc_tiles(8, P, FH, "wvTt")
        owT_t = alloc_tiles(4, P, T, "owTt")
        relM_t = {k: alloc_tiles(8, P, T, f"rMt{k}") for k in relM_d}
        if mask_on:
            expm_t = alloc_tiles(8, P, T, "emt")

        qb_t = persist.tile([P, 4], f32, name="qbt", tag="qbt")
        kb_t = persist.tile([P, 4], f32, name="kbt", tag="kbt")
        vbb_t = persist.tile([P, FH], f32, name="vbbt", tag="vbbt")
        obb_t = persist.tile([P, E], f32, name="obbt", tag="obbt")

        # interleaved: per e-chunk, everything QKV needs for that chunk
        for ec in range(8):
            load_chunk(xT_t, xT_d, P, ec)
            load_chunk(wqT_t, wqT_d, P, ec)
            load_chunk(wkT_t, wkT_d, P, ec)
            load_chunk(wvT_t, wvT_d, P, ec)
        dma_in(qb_t[:], qb_d[:])
        dma_in(kb_t[:], kb_d[:])
        dma_in(vbb_t[:], vbb_d[:])
        if mask_on:
            for i in range(8):
                load_chunk(expm_t, expm_d, P, i)
        for k, d in relM_d.items():
            for i in range(8):
                load_chunk(relM_t[k], d, P, i)
        for i in range(4):
            load_chunk(owT_t, owT_d, P, i)
        dma_in(obb_t[:], obb_d[:])

        ones64 = persist.tile([P, 64], bf, name="ones64", tag="ones64")
        nc.vector.memset(ones64[:], 1.0)
        warm_rhs = persist.tile([P, 512], bf, name="warm_rhs", tag="warm_rhs")
        nc.vector.memset(warm_rhs[:], 0.5)

        qT_t = [persist.tile([P, T], bf, name=f"qTs{p}", tag=f"qTs{p}")
                for p in range(4)]
        kT_t = [persist.tile([P, T], bf, name=f"kTs{p}", tag=f"kTs{p}")
                for p in range(4)]
        v_sb = [persist.tile([P, FH], bf, name=f"vs{s}", tag=f"vs{s}")
                for s in range(8)]
        oT_sb = [persist.tile([P, T], bf, name=f"oTs{p}", tag=f"oTs{p}")
                 for p in range(4)]

        # ---------------- projections (column-tiled 128x64) ----------
        # All projection matmuls run in 128x64 column-tiled mode -- the
        # same PE mode as den/attn@v -- so the scheduler can interleave
        # them without mode-switch pipeline flushes, and adjacent T0/T1
        # instructions dual-issue. Per phase the two concurrent halves
        # accumulate in different PSUM banks.
        def emit_qk_pair(pool, p, tag):
            for (w_t, b_t, dst) in ((wqT_t, qb_t, qT_t[p]),
                                    (wkT_t, kb_t, kT_t[p])):
                ps = pool.tile([P, T], f32, name="qk_ps", tag=tag, bufs=2)
                for phase in range(2):
                    for ec in range(8):
                        lo = w_t[ec][:, P * p:P * p + 64]
                        hi = w_t[ec][:, P * p + 64:P * (p + 1)]
                        asl = slice(512 * phase, 512 * (phase + 1))
                        bsl = slice(512 * (1 - phase), 512 * (2 - phase))
                        nc.tensor.matmul(
                            ps[0:64, asl], lhsT=lo, rhs=xT_t[ec][:, asl],
                            start=(ec == 0), stop=(ec == 7))
                        nc.tensor.matmul(
                            ps[64:128, bsl], lhsT=hi, rhs=xT_t[ec][:, bsl],
                            start=(ec == 0), stop=(ec == 7))
                nc.vector.tensor_scalar_add(dst[:], ps[:], b_t[:, p:p + 1])

        with tc.tile_pool(name="qkv0_ps", bufs=1, space="PSUM") as q0_pool:
            # warm the PE (HAM un-throttles after ~3.5us of activity) while
            # the input DMAs stream in; col-tiled like everything after.
            wps = q0_pool.tile([P, T], f32, name="wps", tag="v_ps", bufs=2)
            for i in range(18):
                nc.tensor.matmul(wps[0:64, 0:512], lhsT=ones64[:, 0:64],
                                 rhs=warm_rhs[:], start=True, stop=True)
                nc.tensor.matmul(wps[64:128, 512:1024], lhsT=ones64[:, 0:64],
                                 rhs=warm_rhs[:], start=True, stop=True)

            emit_qk_pair(q0_pool, 0, "qk0_ps")
            for s in range(0, 8, 2):
                ps = q0_pool.tile([P, T], f32, name="v_ps", tag="v_ps",
                                  bufs=2)
                for phase in range(2):
                    ra = slice(0, 64) if phase == 0 else slice(64, 128)
                    rb = slice(64, 128) if phase == 0 else slice(0, 64)
                    for ec in range(8):
                        nc.tensor.matmul(
                            ps[ra, 0:512],
                            lhsT=xT_t[ec][:, P * s + ra.start:
                                          P * s + ra.stop],
                            rhs=wvT_t[ec][:],
                            start=(ec == 0), stop=(ec == 7))
                        nc.tensor.matmul(
                            ps[rb, 512:1024],
                            lhsT=xT_t[ec][:, P * (s + 1) + rb.start:
                                          P * (s + 1) + rb.stop],
                            rhs=wvT_t[ec][:],
                            start=(ec == 0), stop=(ec == 7))
                nc.vector.tensor_add(v_sb[s][:], ps[:, 0:512], vbb_t[:])
                nc.vector.tensor_add(v_sb[s + 1][:], ps[:, 512:1024],
                                     vbb_t[:])

        # ---------------- attention (with lazy Q/K for later pairs) ----
        with tc.tile_pool(name="s_ps", bufs=1, space="PSUM") as s_pool, \
             tc.tile_pool(name="w_ps", bufs=1, space="PSUM") as w_pool, \
             tc.tile_pool(name="e_sb", bufs=1) as e_pool, \
             tc.tile_pool(name="bc_sb", bufs=1) as bc_pool:
            for p in range(4):
                hA, hB = 2 * p, 2 * p + 1
                eT = {hA: [], hB: []}
                for sc in range(8):
                    sA = s_pool.tile([P, T], f32, name="s_A", tag="s_ps",
                                     bufs=2)
                    sB = s_pool.tile([P, T], f32, name="s_B", tag="s_ps",
                                     bufs=2)
                    for th in range(2):
                        tsl = slice(512 * th, 512 * (th + 1))
                        nc.tensor.matmul(
                            sA[:, tsl],
                            lhsT=kT_t[p][0:64, P * sc:P * (sc + 1)],
                            rhs=qT_t[p][0:64, tsl],
                            start=True, stop=True)
                    for th in range(2):
                        tsl = slice(512 * th, 512 * (th + 1))
                        nc.tensor.matmul(
                            sB[:, tsl],
                            lhsT=kT_t[p][64:128, P * sc:P * (sc + 1)],
                            rhs=qT_t[p][64:128, tsl],
                            start=True, stop=True)
                    eA = e_pool.tile([P, T], bf, name="e_t", tag="e_t",
                                     bufs=20)
                    nc.scalar.activation(eA[:], sA[:], Exp)
                    eB = e_pool.tile([P, T], bf, name="e_t", tag="e_t",
                                     bufs=20)
                    nc.scalar.activation(eB[:], sB[:], Exp)
                    if mask_on:
                        nc.vector.tensor_mul(eA[:], eA[:], expm_t[sc][:])
                        nc.vector.tensor_mul(eB[:], eB[:], expm_t[sc][:])
                    eT[hA].append(eA)
                    eT[hB].append(eB)

                # denominators (from pre-relation E): both heads in one
                # PSUM tile, row-sums replicated over 64 partitions; per
                # phase the heads target different t-half banks.
                den = w_pool.tile([P, T], f32, name="den", tag="w_ps",
                                  bufs=2)
                for phase in range(2):
                    asl = slice(512 * phase, 512 * (phase + 1))
                    bsl = slice(512 * (1 - phase), 512 * (2 - phase))
                    for sc in range(8):
                        nc.tensor.matmul(
                            den[0:64, asl], lhsT=ones64[:],
                            rhs=eT[hA][sc][:, asl],
                            start=(sc == 0), stop=(sc == 7))
                        nc.tensor.matmul(
                            den[64:128, bsl], lhsT=ones64[:],
                            rhs=eT[hB][sc][:, bsl],
                            start=(sc == 0), stop=(sc == 7))

                # per-slot relation multipliers (in place, after den)
                for lh in (hA, hB):
                    if slot_flags[lh]:
                        for sc in range(8):
                            nc.vector.tensor_mul(eT[lh][sc][:],
                                                 eT[lh][sc][:],
                                                 relM_t[lh][sc][:])

                # attn @ v -> outT [2x64 f rows, t]
                ops = w_pool.tile([P, T], f32, name="ops", tag="w_ps",
                                  bufs=2)
                for phase in range(2):
                    asl = slice(512 * phase, 512 * (phase + 1))
                    bsl = slice(512 * (1 - phase), 512 * (2 - phase))
                    for sc in range(8):
                        nc.tensor.matmul(
                            ops[0:64, asl],
                            lhsT=v_sb[sc][:, 64 * hA:64 * hA + 64],
                            rhs=eT[hA][sc][:, asl],
                            start=(sc == 0), stop=(sc == 7))
                        nc.tensor.matmul(
                            ops[64:128, bsl],
                            lhsT=v_sb[sc][:, 64 * hB:64 * hB + 64],
                            rhs=eT[hB][sc][:, bsl],
                            start=(sc == 0), stop=(sc == 7))

                bc = bc_pool.tile([P, T], f32, name="bc", tag="bc", bufs=2)
                nc.vector.reciprocal_approx_fast(bc[:], den[:])
                nc.vector.tensor_mul(oT_sb[p][:], ops[:], bc[:])

                if p + 1 < 4:
                    emit_qk_pair(w_pool, p + 1, "w_ps")

        # ---------------- output projection ----------------
        with tc.tile_pool(name="y_ps", bufs=1, space="PSUM") as y_pool, \
             tc.tile_pool(name="y_sb", bufs=1) as ysb_pool:
            for tcn in range(8):
                yps = y_pool.tile([P, E], f32, name="yps", tag="yps", bufs=2)
                for eh in range(2):
                    esl = slice(512 * eh, 512 * (eh + 1))
                    for fc in range(4):
                        nc.tensor.matmul(
                            yps[:, esl],
                            lhsT=oT_sb[fc][:, P * tcn:P * (tcn + 1)],
                            rhs=owT_t[fc][:, esl],
                            start=(fc == 0), stop=(fc == 3))
                ysb = ysb_pool.tile([P, E], f32, name="ysb", tag="ysb",
                                    bufs=2)
                nc.vector.tensor_add(ysb[:], yps[:], obb_t[:])
                dma_engines[tcn % len(dma_engines)].dma_start(
                    y_d[P * tcn:P * (tcn + 1), :], ysb[:])

    nc.compile()
    return nc


def _get_program(mask_on, slot_flags):
    key = (mask_on, slot_flags)
    if key not in _PROGS:
        _PROGS[key] = _build_program(mask_on, slot_flags)
    return _PROGS[key]


def _prep_inputs(inputs):
    hs = np.asarray(inputs["hidden_states"], dtype=np.float32)
    am = np.asarray(inputs["attention_mask"], dtype=np.float32)
    rel = np.asarray(inputs["relation_inputs"])
    hm = np.asarray(inputs["heads_mask"], dtype=np.float32)
    q_w = np.asarray(inputs["q_w"], dtype=np.float32)
    q_b = np.asarray(inputs["q_b"], dtype=np.float32)
    k_w = np.asarray(inputs["k_w"], dtype=np.float32)
    k_b = np.asarray(inputs["k_b"], dtype=np.float32)
    v_w = np.asarray(inputs["v_w"], dtype=np.float32)
    v_b = np.asarray(inputs["v_b"], dtype=np.float32)
    o_w = np.asarray(inputs["o_w"], dtype=np.float32)
    o_b = np.asarray(inputs["o_b"], dtype=np.float32)

    mask_on = bool(np.any(am != 0.0))
    slot_flags = tuple(
        k == 0 or bool(np.any(hm[[k, 8 + k]] != 0.0)) for k in range(8))

    relbinT = [(rel[b] > 0).T.astype(np.float32) for b in range(B)]
    if mask_on:
        expmT = [np.exp(am[b, 0]).T.astype(BF16) for b in range(B)]

    in_maps = []
    for c in range(N_CORES):
        b, g = c // 2, c % 2
        sl = slice(FH * g, FH * (g + 1))
        im = {
            "xT": np.ascontiguousarray(hs[b].T).astype(BF16),
            "wqT": np.ascontiguousarray((q_w[sl] * SCALING).T).astype(BF16),
            "wkT": np.ascontiguousarray(k_w[sl].T).astype(BF16),
            "wvT": np.ascontiguousarray(v_w[sl].T).astype(BF16),
            "owT": np.ascontiguousarray(o_w[:, sl].T).astype(BF16),
            "qb": np.ascontiguousarray(
                (q_b[sl] * SCALING).reshape(4, P).T).astype(np.float32),
            "kb": np.ascontiguousarray(
                k_b[sl].reshape(4, P).T).astype(np.float32),
            "vbb": np.ascontiguousarray(
                np.broadcast_to(v_b[sl], (P, FH))).astype(np.float32),
            "obb": (np.ascontiguousarray(np.broadcast_to(o_b, (P, E)))
                    .astype(np.float32) if g == 0
                    else np.zeros((P, E), np.float32)),
        }
        for k in range(8):
            if slot_flags[k]:
                hmv = float(hm[8 * g + k])
                m = (1.0 - hmv) + hmv * relbinT[b]
                im[f"relM{k}"] = m.astype(BF16)
        if mask_on:
            im["expmaskT"] = expmT[b]
        in_maps.append(im)
    return mask_on, slot_flags, in_maps


def _gather(results):
    out = np.empty((B, T, E), dtype=np.float32)
    for b in range(B):
        out[b] = results[2 * b]["y"] + results[2 * b + 1]["y"]
    return out


def run_sharded(inputs, trace=False, trace_kwargs=None):
    from concourse.bass_utils import run_bass_kernel_spmd

    mask_on, slot_flags, in_maps = _prep_inputs(inputs)
    nc = _get_program(mask_on, slot_flags)
    res = run_bass_kernel_spmd(nc, in_maps, list(range(N_CORES)),
                               trace=trace, **(trace_kwargs or {}))
    return _gather(res.results), res


def kernel(**inputs):
    out, _ = run_sharded(inputs)
    return out


# revision 10
# speedup vs baseline: 1.2066x; 1.2066x over previous
"""Trainium2 Bass kernel for BART custom-mask attention.

Problem: B=4, T=S=1024, E=1024, H=16 heads, D=64.
  q = (hs @ q_w.T + q_b) * D**-0.5 ; k/v analogous
  scores = q k^T + attention_mask ; attn = softmax(scores)
  attn(head h) *= (1-hm[h]) + hm[h]*(relation_inputs>0)   (no renorm)
  out = (attn @ v) @ o_w.T + o_b

Sharding: 8 cores = batch (4) x head-group (2, 8 heads each). Each core
computes a 512-feature slice of the attention output and projects it
through the matching o_w columns; the host sums the two half-partials
per batch.

On-core layout (bf16 compute, fp32 accumulation):
  - projections consume host-pre-transposed xT/wT so the contraction dim
    (e) is always on partitions; q/k are produced transposed [f, t],
    v natural [s, f].
  - scoresT [s, t] per head via 64x128 row-tiled PE (two heads of a pair
    run on independent half-arrays T0/T8).
  - exp on ScalarE straight out of PSUM -> bf16 E tiles.
  - denominators = ones[128,64]^T @ E via 128x64 column-tiled PE: both
    heads of a pair land in one [128,1024] PSUM tile with the per-head
    row sums replicated across 64 partitions -- exactly the broadcast the
    normalization multiply needs.
  - attn@v column-tiled the same way -> outT [f(128), t] PSUM per pair;
    one reciprocal_approx_fast + one tensor_mul does normalize+copyback.
  - output projection contracts f (on partitions) -> y [t, e] natural.
"""

import os
import sys

import numpy as np

for _p in ("/opt/trn_rl_repo", "/root/.axon_site/_ro/trn_rl_repo"):
    if os.path.isdir(_p) and _p not in sys.path:
        sys.path.insert(0, _p)
        break

import ml_dtypes

B, T, E, H = 4, 1024, 1024, 16
D = E // H
SCALING = D ** -0.5
N_CORES = 8
FH = 512          # features per core (8 heads x 64)
P = 128
BF16 = ml_dtypes.bfloat16

_PROGS = {}


def _build_program(mask_on, slot_flags):
    import concourse.tile as tile
    from concourse import bacc, mybir
    from contextlib import ExitStack

    bf = mybir.dt.bfloat16
    f32 = mybir.dt.float32
    Exp = mybir.ActivationFunctionType.Exp

    nc = bacc.Bacc("TRN2", target_bir_lowering=False, debug=False,
                   num_devices=N_CORES)

    xT_d = nc.declare_dram_parameter("xT", [E, T], bf, isOutput=False)
    wqT_d = nc.declare_dram_parameter("wqT", [E, FH], bf, isOutput=False)
    wkT_d = nc.declare_dram_parameter("wkT", [E, FH], bf, isOutput=False)
    wvT_d = nc.declare_dram_parameter("wvT", [E, FH], bf, isOutput=False)
    owT_d = nc.declare_dram_parameter("owT", [FH, E], bf, isOutput=False)
    qb_d = nc.declare_dram_parameter("qb", [P, 4], f32, isOutput=False)
    kb_d = nc.declare_dram_parameter("kb", [P, 4], f32, isOutput=False)
    vbb_d = nc.declare_dram_parameter("vbb", [P, FH], f32, isOutput=False)
    obb_d = nc.declare_dram_parameter("obb", [P, E], f32, isOutput=False)
    relM_d = {}
    for k in range(8):
        if slot_flags[k]:
            relM_d[k] = nc.declare_dram_parameter(f"relM{k}", [T, T], bf,
                                                  isOutput=False)
    if mask_on:
        expm_d = nc.declare_dram_parameter("expmaskT", [T, T], bf,
                                           isOutput=False)
    y_d = nc.declare_dram_parameter("y", [T, E], f32, isOutput=True)

    with tile.TileContext(nc) as tc, ExitStack() as ctx:
        persist = ctx.enter_context(tc.tile_pool(name="persist", bufs=1))

        # Spread input loads across the three DMA-capable engines (sync/
        # scalar HWDGE + gpsimd SWDGE) so they don't serialize on one
        # queue, and order them so the QKV-critical chunks land first.
        dma_engines = [nc.sync, nc.scalar, nc.gpsimd]
        dma_rr = [0]

        def dma_in(out_ap, in_ap):
            eng = dma_engines[dma_rr[0] % len(dma_engines)]
            dma_rr[0] += 1
            eng.dma_start(out_ap, in_ap)

        def alloc_tiles(n, rows, cols, nm, dt=bf):
            return [persist.tile([rows, cols], dt, name=f"{nm}{i}",
                                 tag=f"{nm}{i}") for i in range(n)]

        def load_chunk(ts, dram, rows, i):
            dma_in(ts[i][:], dram[rows * i:rows * (i + 1), :])

        xT_t = alloc_tiles(8, P, T, "xTt")
        wqT_t = alloc_tiles(8, P, FH, "wqTt")
        wkT_t = alloc_tiles(8, P, FH, "wkTt")
        wvT_t = alloc_tiles(8, P, FH, "wvTt")
        owT_t = alloc_tiles(4, P, T, "owTt")
        relM_t = {k: alloc_tiles(8, P, T, f"rMt{k}") for k in relM_d}
        if mask_on:
            expm_t = alloc_tiles(8, P, T, "emt")

        qb_t = persist.tile([P, 4], f32, name="qbt", tag="qbt")
        kb_t = persist.tile([P, 4], f32, name="kbt", tag="kbt")
        vbb_t = persist.tile([P, FH], f32, name="vbbt", tag="vbbt")
        obb_t = persist.tile([P, E], f32, name="obbt", tag="obbt")

        # interleaved: per e-chunk, everything QKV needs for that chunk
        for ec in range(8):
            load_chunk(xT_t, xT_d, P, ec)
            load_chunk(wqT_t, wqT_d, P, ec)
            load_chunk(wkT_t, wkT_d, P, ec)
            load_chunk(wvT_t, wvT_d, P, ec)
        dma_in(qb_t[:], qb_d[:])
        dma_in(kb_t[:], kb_d[:])
        dma_in(vbb_t[:], vbb_d[:])
        if mask_on:
            for i in range(8):
                load_chunk(expm_t, expm_d, P, i)
        for k, d in relM_d.items():
            for i in range(8):
                load_chunk(relM_t[k], d, P, i)
        for i in range(4):
            load_chunk(owT_t, owT_d, P, i)
        dma_in(obb_t[:], obb_d[:])

        ones64 = persist.tile([P, 64], bf, name="ones64", tag="ones64")
        nc.vector.memset(ones64[:], 1.0)
        warm_rhs = persist.tile([P, 512], bf, name="warm_rhs", tag="warm_rhs")
        nc.vector.memset(warm_rhs[:], 0.5)

        qT_t = [persist.tile([P, T], bf, name=f"qTs{p}", tag=f"qTs{p}")
                for p in range(4)]
        kT_t = [persist.tile([P, T], bf, name=f"kTs{p}", tag=f"kTs{p}")
                for p in range(4)]
        v_sb = [persist.tile([P, FH], bf, name=f"vs{s}", tag=f"vs{s}")
                for s in range(8)]
        oT_sb = [persist.tile([P, T], bf, name=f"oTs{p}", tag=f"oTs{p}")
                 for p in range(4)]

        # ---------------- projections (column-tiled 128x64) ----------
        # All projection matmuls run in 128x64 column-tiled mode -- the
        # same PE mode as den/attn@v -- so the scheduler can interleave
        # them without mode-switch pipeline flushes, and adjacent T0/T1
        # instructions dual-issue. Per phase the two concurrent halves
        # accumulate in different PSUM banks.
        def emit_qk_pair(pool, p, tag):
            for (w_t, b_t, dst) in ((wqT_t, qb_t, qT_t[p]),
                                    (wkT_t, kb_t, kT_t[p])):
                ps = pool.tile([P, T], f32, name="qk_ps", tag=tag, bufs=2)
                for phase in range(2):
                    for ec in range(8):
                        lo = w_t[ec][:, P * p:P * p + 64]
                        hi = w_t[ec][:, P * p + 64:P * (p + 1)]
                        asl = slice(512 * phase, 512 * (phase + 1))
                        bsl = slice(512 * (1 - phase), 512 * (2 - phase))
                        nc.tensor.matmul(
                            ps[0:64, asl], lhsT=lo, rhs=xT_t[ec][:, asl],
                            start=(ec == 0), stop=(ec == 7))
                        nc.tensor.matmul(
                            ps[64:128, bsl], lhsT=hi, rhs=xT_t[ec][:, bsl],
                            start=(ec == 0), stop=(ec == 7))
                nc.vector.tensor_scalar_add(dst[:], ps[:], b_t[:, p:p + 1])

        with tc.tile_pool(name="qkv0_ps", bufs=1, space="PSUM") as q0_pool:
            # warm the PE (HAM un-throttles after ~3.5us of activity) while
            # the input DMAs stream in; col-tiled like everything after.
            wps = q0_pool.tile([P, T], f32, name="wps", tag="v_ps", bufs=2)
            for i in range(18):
                nc.tensor.matmul(wps[0:64, 0:512], lhsT=ones64[:, 0:64],
                                 rhs=warm_rhs[:], start=True, stop=True)
                nc.tensor.matmul(wps[64:128, 512:1024], lhsT=ones64[:, 0:64],
                                 rhs=warm_rhs[:], start=True, stop=True)

            emit_qk_pair(q0_pool, 0, "qk0_ps")
            for s in range(0, 8, 2):
                ps = q0_pool.tile([P, T], f32, name="v_ps", tag="v_ps",
                                  bufs=2)
                for phase in range(2):
                    ra = slice(0, 64) if phase == 0 else slice(64, 128)
                    rb = slice(64, 128) if phase == 0 else slice(0, 64)
                    for ec in range(8):
                        nc.tensor.matmul(
                            ps[ra, 0:512],
                            lhsT=xT_t[ec][:, P * s + ra.start:
                                          P * s + ra.stop],
                            rhs=wvT_t[ec][:],
                            start=(ec == 0), stop=(ec == 7))
                        nc.tensor.matmul(
                            ps[rb, 512:1024],
                            lhsT=xT_t[ec][:, P * (s + 1) + rb.start:
                                          P * (s + 1) + rb.stop],
                            rhs=wvT_t[ec][:],
                            start=(ec == 0), stop=(ec == 7))
                nc.vector.tensor_add(v_sb[s][:], ps[:, 0:512], vbb_t[:])
                nc.vector.tensor_add(v_sb[s + 1][:], ps[:, 512:1024],
                                     vbb_t[:])

        # ------- attention: software-pipelined by one pair -----------
        # Iteration p emits: scores(p) [row-mode] -> exp(p) on ACT;
        # qk(p+1) [col-mode]; attn@v(p-1) [col] (its exp finished a full
        # iteration ago -- no PE stall); den(p) [col] trailing exp(p);
        # slot-muls(p); recip/copyback(p-1). PE never waits long on ACT,
        # so HAM stays un-throttled.
        with tc.tile_pool(name="s_ps", bufs=1, space="PSUM") as s_pool, \
             tc.tile_pool(name="w_ps", bufs=1, space="PSUM") as w_pool, \
             tc.tile_pool(name="e_sb", bufs=1) as e_pool, \
             tc.tile_pool(name="bc_sb", bufs=1) as bc_pool:
            eT = {}
            den_t = {}
            ops_t = {}
            bc_t = {}

            def emit_scores_exp(p):
                hA, hB = 2 * p, 2 * p + 1
                eT[hA] = []
                eT[hB] = []
                for sc in range(8):
                    sA = s_pool.tile([P, T], f32, name="s_A", tag="s_ps",
                                     bufs=2)
                    sB = s_pool.tile([P, T], f32, name="s_B", tag="s_ps",
                                     bufs=2)
                    for th in range(2):
                        tsl = slice(512 * th, 512 * (th + 1))
                        nc.tensor.matmul(
                            sA[:, tsl],
                            lhsT=kT_t[p][0:64, P * sc:P * (sc + 1)],
                            rhs=qT_t[p][0:64, tsl],
                            start=True, stop=True)
                    for th in range(2):
                        tsl = slice(512 * th, 512 * (th + 1))
                        nc.tensor.matmul(
                            sB[:, tsl],
                            lhsT=kT_t[p][64:128, P * sc:P * (sc + 1)],
                            rhs=qT_t[p][64:128, tsl],
                            start=True, stop=True)
                    eA = e_pool.tile([P, T], bf, name="e_t", tag="e_t",
                                     bufs=32)
                    nc.scalar.activation(eA[:], sA[:], Exp)
                    eB = e_pool.tile([P, T], bf, name="e_t", tag="e_t",
                                     bufs=32)
                    nc.scalar.activation(eB[:], sB[:], Exp)
                    if mask_on:
                        nc.vector.tensor_mul(eA[:], eA[:], expm_t[sc][:])
                        nc.vector.tensor_mul(eB[:], eB[:], expm_t[sc][:])
                    eT[hA].append(eA)
                    eT[hB].append(eB)

            def emit_den_and_muls(p):
                hA, hB = 2 * p, 2 * p + 1
                den = w_pool.tile([P, T], f32, name="den", tag="w_ps",
                                  bufs=2)
                den_t[p] = den
                for phase in range(2):
                    asl = slice(512 * phase, 512 * (phase + 1))
                    bsl = slice(512 * (1 - phase), 512 * (2 - phase))
                    for sc in range(8):
                        nc.tensor.matmul(
                            den[0:64, asl], lhsT=ones64[:],
                            rhs=eT[hA][sc][:, asl],
                            start=(sc == 0), stop=(sc == 7))
                        nc.tensor.matmul(
                            den[64:128, bsl], lhsT=ones64[:],
                            rhs=eT[hB][sc][:, bsl],
                            start=(sc == 0), stop=(sc == 7))
                for lh in (hA, hB):
                    if slot_flags[lh]:
                        for sc in range(8):
                            nc.vector.tensor_mul(eT[lh][sc][:],
                                                 eT[lh][sc][:],
                                                 relM_t[lh][sc][:])

            def emit_recip(p):
                bc = bc_pool.tile([P, T], f32, name="bc", tag="bc", bufs=2)
                bc_t[p] = bc
                nc.vector.reciprocal_approx_fast(bc[:], den_t[p][:])

            def emit_av(p):
                hA, hB = 2 * p, 2 * p + 1
                ops = w_pool.tile([P, T], f32, name="ops", tag="w_ps",
                                  bufs=2)
                ops_t[p] = ops
                for phase in range(2):
                    asl = slice(512 * phase, 512 * (phase + 1))
                    bsl = slice(512 * (1 - phase), 512 * (2 - phase))
                    for sc in range(8):
                        nc.tensor.matmul(
                            ops[0:64, asl],
                            lhsT=v_sb[sc][:, 64 * hA:64 * hA + 64],
                            rhs=eT[hA][sc][:, asl],
                            start=(sc == 0), stop=(sc == 7))
                        nc.tensor.matmul(
                            ops[64:128, bsl],
                            lhsT=v_sb[sc][:, 64 * hB:64 * hB + 64],
                            rhs=eT[hB][sc][:, bsl],
                            start=(sc == 0), stop=(sc == 7))
                for sc in range(8):
                    eT[hA][sc] = None
                    eT[hB][sc] = None

            def emit_copyback(p):
                nc.vector.tensor_mul(oT_sb[p][:], ops_t[p][:], bc_t[p][:])

            for p in range(5):
                if p < 4:
                    emit_scores_exp(p)
                if p >= 1:
                    emit_recip(p - 1)
                if p + 1 < 4:
                    emit_qk_pair(w_pool, p + 1, "w_ps")
                if p >= 1:
                    emit_av(p - 1)
                if p < 4:
                    emit_den_and_muls(p)
                if p >= 1:
                    emit_copyback(p - 1)

        # ---------------- output projection ----------------
        with tc.tile_pool(name="y_ps", bufs=1, space="PSUM") as y_pool, \
             tc.tile_pool(name="y_sb", bufs=1) as ysb_pool:
            for tcn in range(8):
                yps = y_pool.tile([P, E], f32, name="yps", tag="yps", bufs=2)
                for eh in range(2):
                    esl = slice(512 * eh, 512 * (eh + 1))
                    for fc in range(4):
                        nc.tensor.matmul(
                            yps[:, esl],
                            lhsT=oT_sb[fc][:, P * tcn:P * (tcn + 1)],
                            rhs=owT_t[fc][:, esl],
                            start=(fc == 0), stop=(fc == 3))
                ysb = ysb_pool.tile([P, E], f32, name="ysb", tag="ysb",
                                    bufs=2)
                nc.vector.tensor_add(ysb[:], yps[:], obb_t[:])
                dma_engines[tcn % len(dma_engines)].dma_start(
                    y_d[P * tcn:P * (tcn + 1), :], ysb[:])

    nc.compile()
    return nc


def _get_program(mask_on, slot_flags):
    key = (mask_on, slot_flags)
    if key not in _PROGS:
        _PROGS[key] = _build_program(mask_on, slot_flags)
    return _PROGS[key]


def _prep_inputs(inputs):
    hs = np.asarray(inputs["hidden_states"], dtype=np.float32)
    am = np.asarray(inputs["attention_mask"], dtype=np.float32)
    rel = np.asarray(inputs["relation_inputs"])
    hm = np.asarray(inputs["heads_mask"], dtype=np.float32)
    q_w = np.asarray(inputs["q_w"], dtype=np.float32)
    q_b = np.asarray(inputs["q_b"], dtype=np.float32)
    k_w = np.asarray(inputs["k_w"], dtype=np.float32)
    k_b = np.asarray(inputs["k_b"], dtype=np.float32)
    v_w = np.asarray(inputs["v_w"], dtype=np.float32)
    v_b = np.asarray(inputs["v_b"], dtype=np.float32)
    o_w = np.asarray(inputs["o_w"], dtype=np.float32)
    o_b = np.asarray(inputs["o_b"], dtype=np.float32)

    mask_on = bool(np.any(am != 0.0))
    slot_flags = tuple(
        k == 0 or bool(np.any(hm[[k, 8 + k]] != 0.0)) for k in range(8))

    relbinT = [(rel[b] > 0).T.astype(np.float32) for b in range(B)]
    if mask_on:
        expmT = [np.exp(am[b, 0]).T.astype(BF16) for b in range(B)]

    in_maps = []
    for c in range(N_CORES):
        b, g = c // 2, c % 2
        sl = slice(FH * g, FH * (g + 1))
        im = {
            "xT": np.ascontiguousarray(hs[b].T).astype(BF16),
            "wqT": np.ascontiguousarray((q_w[sl] * SCALING).T).astype(BF16),
            "wkT": np.ascontiguousarray(k_w[sl].T).astype(BF16),
            "wvT": np.ascontiguousarray(v_w[sl].T).astype(BF16),
            "owT": np.ascontiguousarray(o_w[:, sl].T).astype(BF16),
            "qb": np.ascontiguousarray(
                (q_b[sl] * SCALING).reshape(4, P).T).astype(np.float32),
            "kb": np.ascontiguousarray(
                k_b[sl].reshape(4, P).T).astype(np.float32),
            "vbb": np.ascontiguousarray(
                np.broadcast_to(v_b[sl], (P, FH))).astype(np.float32),
            "obb": (np.ascontiguousarray(np.broadcast_to(o_b, (P, E)))
                    .astype(np.float32) if g == 0
                    else np.zeros((P, E), np.float32)),
        }
        for k in range(8):
            if slot_flags[k]:
                hmv = float(hm[8 * g + k])
                m = (1.0 - hmv) + hmv * relbinT[b]
                im[f"relM{k}"] = m.astype(BF16)
        if mask_on:
            im["expmaskT"] = expmT[b]
        in_maps.append(im)
    return mask_on, slot_flags, in_maps


def _gather(results):
    out = np.empty((B, T, E), dtype=np.float32)
    for b in range(B):
        out[b] = results[2 * b]["y"] + results[2 * b + 1]["y"]
    return out


def run_sharded(inputs, trace=False, trace_kwargs=None):
    from concourse.bass_utils import run_bass_kernel_spmd

    mask_on, slot_flags, in_maps = _prep_inputs(inputs)
    nc = _get_program(mask_on, slot_flags)
    res = run_bass_kernel_spmd(nc, in_maps, list(range(N_CORES)),
                               trace=trace, **(trace_kwargs or {}))
    return _gather(res.results), res


def kernel(**inputs):
    out, _ = run_sharded(inputs)
    return out
